# revision 1
# baseline (speedup 1.0000x reference)
"""Trainium2 Bass kernel for CircleProjectionLayer (ball projection, r=1).

out = center + d * min(1, 1/||d||),  d = x - center,  shapes [8388608, 3] f32.

Sharding: pure data parallel — batch split 8 ways, one shard per NeuronCore.
Per-core layout: the [1048576, 3] shard is viewed flat as [128, 24576] so each
SBUF partition holds 8192 complete (x,y,z) rows contiguously; chunks of
W floats per partition stream through SBUF.

Engine split (all fp32):
  DVE   : d = x-c; s01 = dx^2+dy^2; s = max(s01,eps)+dz^2 (fused
          scalar_tensor_tensor); m_k = d_k * scale (x3, into the sq tile)
  ACT   : sq = Square(d); l = Ln(s); rl = Relu(l); scale = Exp(-0.5*rl)
          (one pre-placed table load: natural_log_exp_and_others covers all 4;
           exp(-0.5*relu(ln(s))) == min(1, rsqrt(s)) with an exact clamp at 1)
  GPSIMD: out = m + c  (into the d tile; DVE ops are all 1-port TT class, so
          no shared-SBUF-port contention)
  DMA   : x-in + center-in on the SP HWDGE ring; out on the Pool/SWDGE ring so
          late output DMAs can't head-of-line-block the input stream.
"""

import sys

sys.path.insert(0, "/opt/trn_rl_repo")

from contextlib import ExitStack

import numpy as np

import concourse.bass as bass
import concourse.tile as tile
from concourse import bacc, mybir
from concourse.bass_utils import run_bass_kernel_spmd
from concourse.hw_specs import get_activation_tables

F32 = mybir.dt.float32
AF = mybir.ActivationFunctionType
ALU = mybir.AluOpType

B = 8388608
N_CORES = 8
B_CORE = B // N_CORES          # 1048576 rows per core
P = 128
FPP = B_CORE * 3 // P          # 24576 floats per partition

_EPS = 1e-30
_ACT_SET = "natural_log_exp_and_others"


def _preload_act_table(nc):
    """Pre-place one LoadActFuncSet for the set containing Square/Ln/Relu/Exp
    so Bacc.insert_act_table_loads doesn't thrash between greedy choices."""
    tables = list(get_activation_tables(nc.m.arch).keys())
    set_id = tables.index(_ACT_SET)
    inst = mybir.InstLoadActFuncSet(
        name=nc.get_next_instruction_name(), act_func_set_id=set_id, ins=[], outs=[]
    )
    return nc.scalar.add_instruction(inst)


def _build(W=3072, bufs=3, c_ring="sp", out_ring="pool", preload_act=True,
           schedule=None, loop_reps=1, sub_split=0.0, bufs_io=None):
    """`schedule`: optional explicit list of chunk widths (floats/partition,
    each a multiple of 3, summing to FPP). Default: uniform chunks of W.
    `loop_reps`: wrap the whole schedule in a hardware For_i loop (used only
    for benchmarking steady-state HW time via wall-clock deltas)."""
    if schedule is None:
        assert W % 3 == 0 and FPP % W == 0
        schedule = [W] * (FPP // W)
    assert sum(schedule) == FPP and all(w % 3 == 0 for w in schedule)
    W = max(schedule)

    nc = bacc.Bacc("TRN2", target_bir_lowering=False, debug=False)

    x = nc.dram_tensor("x", [B_CORE, 3], F32, kind="ExternalInput")
    c = nc.dram_tensor("center", [B_CORE, 3], F32, kind="ExternalInput")
    o = nc.dram_tensor("out", [B_CORE, 3], F32, kind="ExternalOutput")

    xr = x.ap().rearrange("(p f) c -> p (f c)", p=P)
    cr = c.ap().rearrange("(p f) c -> p (f c)", p=P)
    orr = o.ap().rearrange("(p f) c -> p (f c)", p=P)

    rings = {"sp": nc.sync, "act": nc.scalar, "pool": nc.gpsimd}

    with tile.TileContext(nc) as tc, ExitStack() as ctx:
        if preload_act:
            _preload_act_table(nc)

        io_bufs = bufs_io if bufs_io is not None else bufs
        xp = ctx.enter_context(tc.tile_pool(name="xp", bufs=io_bufs))
        cp = ctx.enter_context(tc.tile_pool(name="cp", bufs=io_bufs))
        dp = ctx.enter_context(tc.tile_pool(name="dp", bufs=bufs))
        sqp = ctx.enter_context(tc.tile_pool(name="sqp", bufs=bufs))
        sp = ctx.enter_context(tc.tile_pool(name="sp", bufs=bufs))

        c_dma = rings[c_ring]
        o_dma = rings[out_ring]

        import contextlib
        loop_cm = tc.For_i(0, loop_reps, 1) if loop_reps > 1 else contextlib.nullcontext()
        with loop_cm:
            emit_body(nc, tc, schedule, W, bufs, c_dma, o_dma,
                      xp, cp, dp, sqp, sp, xr, cr, orr, sub_split=sub_split)

    nc.compile()
    return nc


def emit_body(nc, tc, schedule, W, bufs, c_dma, o_dma, xp, cp, dp, sqp, sp,
              xr, cr, orr, sub_split=0.0):
        off = 0
        for i, w in enumerate(schedule):
            r = w // 3
            xt = xp.tile([P, W], F32, name="xt", tag="xt")[:, :w]
            nc.sync.dma_start(xt[:, :], xr[:, off : off + w])
            ct = cp.tile([P, W], F32, name="ct", tag="ct")[:, :w]
            c_dma.dma_start(ct[:, :], cr[:, off : off + w])

            dt = dp.tile([P, W], F32, name="dt", tag="dt")[:, :w]
            if sub_split > 0.0:
                # Rebalance: tail fraction of the subtract runs on GPSIMD
                # (HW is DVE-bound; Pool has slack).
                w2 = int(w * (1.0 - sub_split)) // 96 * 96
                nc.vector.tensor_sub(dt[:, :w2], xt[:, :w2], ct[:, :w2])
                nc.gpsimd.tensor_sub(dt[:, w2:], xt[:, w2:], ct[:, w2:])
            else:
                nc.vector.tensor_sub(dt[:, :], xt[:, :], ct[:, :])

            sq = sqp.tile([P, W], F32, name="sq", tag="sq")[:, :w]
            nc.scalar.activation(sq[:, :], dt[:, :], AF.Square)

            # Small per-row chain ping-pongs between two [P, R] tiles (A, B).
            sq3 = sq.rearrange("p (r c) -> p r c", c=3)
            ta = sp.tile([P, W // 3], F32, name="ta", tag="ta")[:, :r]
            nc.vector.tensor_add(ta[:, :], sq3[:, :, 0], sq3[:, :, 1])
            tb = sp.tile([P, W // 3], F32, name="tb", tag="tb")[:, :r]
            nc.vector.scalar_tensor_tensor(
                tb[:, :], ta[:, :], _EPS, sq3[:, :, 2], ALU.max, ALU.add
            )
            nc.scalar.activation(ta[:, :], tb[:, :], AF.Ln)
            nc.scalar.activation(tb[:, :], ta[:, :], AF.Relu)
            sc = sp.tile([P, W // 3], F32, name="sc", tag="sc")[:, :r]
            nc.scalar.activation(sc[:, :], tb[:, :], AF.Exp, scale=-0.5)

            # m_k = d_k * scale, into the sq tile (squares are dead now).
            m3 = sq.rearrange("p (r c) -> p r c", c=3)
            d3 = dt.rearrange("p (r c) -> p r c", c=3)
            for k in range(3):
                nc.vector.tensor_mul(m3[:, :, k], d3[:, :, k], sc[:, :])

            # out = m + c, into the d tile (d is dead now).
            nc.gpsimd.tensor_add(dt[:, :], sq[:, :], ct[:, :])

            o_dma.dma_start(orr[:, off : off + w], dt[:, :])
            off += w


_NC = None

# Uniform 3072-wide chunks with a 2x1536 taper at the end: the taper shortens
# the pipeline tail (last chunk's compute chain + out-DMA trail the stream).
_SCHEDULE = [3072] * 7 + [1536] * 2


def _get_nc():
    global _NC
    if _NC is None:
        _NC = _build(schedule=_SCHEDULE)
    return _NC


def kernel(**inputs):
    x = np.asarray(inputs["x"], dtype=np.float32)
    center = np.asarray(inputs["center"], dtype=np.float32)
    assert x.shape == (B, 3) and center.shape == (B, 3)

    xs = x.reshape(N_CORES, B_CORE, 3)
    cs = center.reshape(N_CORES, B_CORE, 3)
    in_maps = [
        {"x": np.ascontiguousarray(xs[i]), "center": np.ascontiguousarray(cs[i])}
        for i in range(N_CORES)
    ]

    nc = _get_nc()
    res = run_bass_kernel_spmd(nc, in_maps, list(range(N_CORES)))
    out = np.concatenate([res.results[i]["out"] for i in range(N_CORES)], axis=0)
    return out.astype(np.float32, copy=False)


if __name__ == "__main__":
    nc = _get_nc()
    print("build ok")



# revision 24
# speedup vs baseline: 62.8233x; 62.8233x over previous
"""Trainium2 Bass kernel for CircleProjectionLayer (ball projection, r=1).

out = center + d * min(1, 1/||d||),  d = x - center,  shapes [8388608, 3] f32.

Sharding: pure data parallel — batch split 8 ways, one shard per NeuronCore.
Per-core layout: the [1048576, 3] shard is viewed flat as [128, 24576] so each
SBUF partition holds 8192 complete (x,y,z) rows contiguously; chunks of
W floats per partition stream through SBUF.

Engine split (all fp32, default algo="rsqrt"):
  DVE   : d = x-c; s01 = dx^2+dy^2; ss = max(s01,eps)+dz^2 (fused stt);
          m = (min(sc,1)) * d (one stt with a stride-0 broadcast of sc);
          out = m + c.  (GPSIMD does NO compute: measured on this part,
          DVE and GPSIMD fully serialize even with independent work, so
          any GPSIMD op time adds 1:1 to DVE time.)
  ACT   : sq = Square(d); sc = Abs_reciprocal_sqrt(ss)  (one pre-placed
          table load: abs_reciprocal_sqrt_and_small covers both; the
          min(sc,1) clamp on DVE keeps inside-ball rows exactly x.)
  DMA   : x-in + center-in on the SP HWDGE ring; out on the ACT HWDGE
          ring so late output DMAs can't head-of-line-block the inputs.
Measured per-core steady state ~127 us vs ~118 us pure-DMA floor
(37.75 MB/core at ~320 GB/s; HBM-per-core spec is ~358 GB/s).
"""

import sys

sys.path.insert(0, "/opt/trn_rl_repo")

from contextlib import ExitStack, nullcontext

import numpy as np

import concourse.bass as bass
import concourse.tile as tile
from concourse import bacc, mybir
from concourse.bass_utils import run_bass_kernel_spmd
from concourse.hw_specs import get_activation_tables

F32 = mybir.dt.float32
AF = mybir.ActivationFunctionType
ALU = mybir.AluOpType

B = 8388608
N_CORES = 8
B_CORE = B // N_CORES          # 1048576 rows per core
P = 128
FPP = B_CORE * 3 // P          # 24576 floats per partition

_EPS = 1e-30
_ACT_SETS = {
    "lnexp": "natural_log_exp_and_others",    # Square/Ln/Relu/Exp
    "lnexp2": "natural_log_exp_and_others",   # Square/Ln/Exp
    "rsqrt": "abs_reciprocal_sqrt_and_small",  # Square/Abs_reciprocal_sqrt
}


def _preload_act_table(nc, algo):
    """Pre-place one LoadActFuncSet for the set containing all needed
    activations so Bacc.insert_act_table_loads doesn't thrash."""
    tables = list(get_activation_tables(nc.m.arch).keys())
    set_id = tables.index(_ACT_SETS[algo])
    inst = mybir.InstLoadActFuncSet(
        name=nc.get_next_instruction_name(), act_func_set_id=set_id, ins=[], outs=[]
    )
    return nc.scalar.add_instruction(inst)


def _build(W=3072, bufs=3, x_ring="sp", c_ring="sp", out_ring="pool",
           preload_act=True, schedule=None, loop_reps=1, sub_split=0.0,
           bufs_io=None, internal_io=False, variant="full", fuse_mul=False,
           add_split=0.0, algo="lnexp"):
    """Build one per-core NEFF.

    schedule: explicit list of chunk widths (floats/partition, each a
      multiple of 3, summing to FPP). Default: uniform chunks of W.
    loop_reps: wrap the whole schedule in a hardware For_i loop (used only
      for benchmarking steady-state HW time via wall-clock deltas).
    internal_io: big tensors become Internal DRAM scratch (garbage data, no
      host transfer) with a tiny seed/tick external pair — timing-only builds.
    variant: "full" | "dma_only" | "in_only" | "out_only" | "compute_only".
    sub_split: tail fraction of the subtract that runs on GPSIMD.
    add_split: fraction of the final add that runs on DVE (rest on GPSIMD).
    fuse_mul: single broadcast-AP mul instead of 3 strided muls.
    """
    if schedule is None:
        assert W % 3 == 0 and FPP % W == 0
        schedule = [W] * (FPP // W)
    assert sum(schedule) == FPP and all(w % 3 == 0 for w in schedule)
    W = max(schedule)

    nc = bacc.Bacc("TRN2", target_bir_lowering=False, debug=False)

    if internal_io:
        x = nc.dram_tensor("x", [B_CORE, 3], F32, kind="Internal")
        c = nc.dram_tensor("center", [B_CORE, 3], F32, kind="Internal")
        o = nc.dram_tensor("out_i", [B_CORE, 3], F32, kind="Internal")
        seed = nc.dram_tensor("seed", [1, 4], F32, kind="ExternalInput")
        tick = nc.dram_tensor("tick", [1, 4], F32, kind="ExternalOutput")
    else:
        x = nc.dram_tensor("x", [B_CORE, 3], F32, kind="ExternalInput")
        c = nc.dram_tensor("center", [B_CORE, 3], F32, kind="ExternalInput")
        o = nc.dram_tensor("out", [B_CORE, 3], F32, kind="ExternalOutput")

    xr = x.ap().rearrange("(p f) c -> p (f c)", p=P)
    cr = c.ap().rearrange("(p f) c -> p (f c)", p=P)
    orr = o.ap().rearrange("(p f) c -> p (f c)", p=P)

    rings = {"sp": nc.sync, "act": nc.scalar, "pool": nc.gpsimd,
             "split": None}

    with tile.TileContext(nc) as tc, ExitStack() as ctx:
        if preload_act and (variant in ("full", "compute_only")
                            or (variant.startswith("eng:") and "act" in variant)):
            _preload_act_table(nc, algo)

        io_bufs = bufs_io if bufs_io is not None else bufs
        xp = ctx.enter_context(tc.tile_pool(name="xp", bufs=io_bufs))
        cp = ctx.enter_context(tc.tile_pool(name="cp", bufs=io_bufs))
        dp = ctx.enter_context(tc.tile_pool(name="dp", bufs=bufs))
        sqp = ctx.enter_context(tc.tile_pool(name="sqp", bufs=bufs))
        sp = ctx.enter_context(tc.tile_pool(name="sp", bufs=bufs))

        if internal_io:
            tinyp = ctx.enter_context(tc.tile_pool(name="tiny", bufs=1))
            tt = tinyp.tile([1, 4], F32, name="tt")
            nc.sync.dma_start(tt[:, :], seed.ap())
            nc.sync.dma_start(tick.ap(), tt[:, :])

        dummy = None
        if variant in ("dma_only", "out_only"):
            dum = ctx.enter_context(tc.tile_pool(name="dum", bufs=1))
            dummy = dum.tile([P, W], F32, name="dummy")
            nc.vector.memset(dummy[:, :], 0.25)

        src = None
        if variant == "compute_only" or variant.startswith("eng:"):
            # Persistent garbage-free sources: written once, read by every
            # chunk — keeps the compute cadence identical to the real kernel
            # without any DMA.
            srcp = ctx.enter_context(tc.tile_pool(name="srcp", bufs=1))
            sx = srcp.tile([P, W], F32, name="sx")
            sy = srcp.tile([P, W], F32, name="sy")
            sa = srcp.tile([P, W // 3], F32, name="sa")
            nc.vector.memset(sx[:, :], 0.75)
            nc.vector.memset(sy[:, :], 0.25)
            nc.vector.memset(sa[:, :], 0.5)
            src = (sx, sy, sa)

        x_dma = rings[x_ring]
        c_dma = rings[c_ring]
        o_dma = rings[out_ring]

        loop_cm = tc.For_i(0, loop_reps, 1) if loop_reps > 1 else nullcontext()
        with loop_cm:
            emit_body(nc, tc, schedule, W, c_dma, o_dma, x_dma,
                      xp, cp, dp, sqp, sp, xr, cr, orr,
                      sub_split=sub_split, variant=variant, dummy=dummy,
                      fuse_mul=fuse_mul, add_split=add_split, algo=algo,
                      src=src)

    nc.compile()
    return nc


def _emit_out(nc, o_dma, dst, srct, w):
    if o_dma is None:
        h = (w // 2) // 4 * 4
        nc.sync.dma_start(dst[:, :h], srct[:, :h])
        nc.scalar.dma_start(dst[:, h:], srct[:, h:])
    else:
        o_dma.dma_start(dst, srct)


def emit_body(nc, tc, schedule, W, c_dma, o_dma, x_dma, xp, cp, dp, sqp, sp,
              xr, cr, orr, sub_split=0.0, variant="full", dummy=None,
              fuse_mul=False, add_split=0.0, algo="lnexp", src=None):
    off = 0
    for i, w in enumerate(schedule):
        r = w // 3
        if variant == "empty":
            continue
        if variant.startswith("eng:"):
            # Engine-isolated throughput: same op shapes as the real chunk,
            # all reads from persistent tiles (no cross-engine deps).
            # Compound names ("eng:dve+gps") run several engines' workloads
            # concurrently with no dependencies between them.
            sx, sy, sa = src
            sx3 = sx.rearrange("p (r c) -> p r c", c=3)
            sy3 = sy.rearrange("p (r c) -> p r c", c=3)
            engs = variant[4:].split("+")
            if "dve" in engs:
                dt = dp.tile([P, W], F32, name="dt", tag="dt")[:, :w]
                nc.vector.tensor_sub(dt[:, :], sx[:, :w], sy[:, :w])
                ta = sp.tile([P, W // 3], F32, name="ta", tag="ta")[:, :r]
                nc.vector.tensor_add(ta[:, :], sx3[:, :r, 0], sx3[:, :r, 1])
                tb = sp.tile([P, W // 3], F32, name="tb", tag="tb")[:, :r]
                nc.vector.scalar_tensor_tensor(
                    tb[:, :], ta[:, :], _EPS, sx3[:, :r, 2], ALU.max, ALU.add
                )
                sq = sqp.tile([P, W], F32, name="sq", tag="sq")[:, :w]
                m3 = sq.rearrange("p (r c) -> p r c", c=3)
                sab = sa[:, :r].broadcast_to([P, r, 3])
                nc.vector.scalar_tensor_tensor(
                    m3[:, :, :], sab, 1.0, sy3[:, :r, :], ALU.min, ALU.mult
                )
            if "act" in engs:
                sq2 = sqp.tile([P, W], F32, name="sq2", tag="sq2")[:, :w]
                nc.scalar.activation(sq2[:, :], sx[:, :w], AF.Square)
                sc = sp.tile([P, W // 3], F32, name="sc", tag="sc")[:, :r]
                nc.scalar.activation(sc[:, :], sa[:, :r], AF.Abs_reciprocal_sqrt)
            if "gps" in engs:
                gt = cp.tile([P, W], F32, name="gt", tag="gt")[:, :w]
                nc.gpsimd.tensor_add(gt[:, :], sx[:, :w], sy[:, :w])
            off += w
            continue
        if variant in ("dma_only", "in_only", "out_only"):
            if variant != "out_only":
                xt = xp.tile([P, W], F32, name="xt", tag="xt")[:, :w]
                x_dma.dma_start(xt[:, :], xr[:, off : off + w])
                ct = cp.tile([P, W], F32, name="ct", tag="ct")[:, :w]
                c_dma.dma_start(ct[:, :], cr[:, off : off + w])
            if variant != "in_only":
                _emit_out(nc, o_dma, orr[:, off : off + w], dummy[:, :w], w)
            off += w
            continue

        if variant == "compute_only":
            xt = src[0][:, :w]
            ct = src[1][:, :w]
        else:
            xt = xp.tile([P, W], F32, name="xt", tag="xt")[:, :w]
            x_dma.dma_start(xt[:, :], xr[:, off : off + w])
            ct = cp.tile([P, W], F32, name="ct", tag="ct")[:, :w]
            c_dma.dma_start(ct[:, :], cr[:, off : off + w])

        dt = dp.tile([P, W], F32, name="dt", tag="dt")[:, :w]
        if sub_split > 0.0:
            # Rebalance: tail fraction of the subtract runs on GPSIMD.
            w2 = int(w * (1.0 - sub_split)) // 96 * 96
            nc.vector.tensor_sub(dt[:, :w2], xt[:, :w2], ct[:, :w2])
            nc.gpsimd.tensor_sub(dt[:, w2:], xt[:, w2:], ct[:, w2:])
        else:
            nc.vector.tensor_sub(dt[:, :], xt[:, :], ct[:, :])

        sq = sqp.tile([P, W], F32, name="sq", tag="sq")[:, :w]
        nc.scalar.activation(sq[:, :], dt[:, :], AF.Square)

        # Small per-row chain ping-pongs between two [P, R] tiles (A, B).
        sq3 = sq.rearrange("p (r c) -> p r c", c=3)
        ta = sp.tile([P, W // 3], F32, name="ta", tag="ta")[:, :r]
        nc.vector.tensor_add(ta[:, :], sq3[:, :, 0], sq3[:, :, 1])
        tb = sp.tile([P, W // 3], F32, name="tb", tag="tb")[:, :r]
        nc.vector.scalar_tensor_tensor(
            tb[:, :], ta[:, :], _EPS, sq3[:, :, 2], ALU.max, ALU.add
        )

        m3 = sq.rearrange("p (r c) -> p r c", c=3)
        d3 = dt.rearrange("p (r c) -> p r c", c=3)

        if algo == "rsqrt":
            # sc = rsqrt(ss); m = min(sc, 1) * d fused in one DVE stt op.
            sc = sp.tile([P, W // 3], F32, name="sc", tag="sc")[:, :r]
            nc.scalar.activation(sc[:, :], tb[:, :], AF.Abs_reciprocal_sqrt)
            scb = sc.broadcast_to([P, r, 3])
            nc.vector.scalar_tensor_tensor(
                m3[:, :, :], scb, 1.0, d3[:, :, :], ALU.min, ALU.mult
            )
        elif algo == "lnexp2":
            # sc = exp(-0.5*ln(ss)) == rsqrt(ss); min fused into the stt mul.
            nc.scalar.activation(ta[:, :], tb[:, :], AF.Ln)
            sc = sp.tile([P, W // 3], F32, name="sc", tag="sc")[:, :r]
            nc.scalar.activation(sc[:, :], ta[:, :], AF.Exp, scale=-0.5)
            scb = sc.broadcast_to([P, r, 3])
            nc.vector.scalar_tensor_tensor(
                m3[:, :, :], scb, 1.0, d3[:, :, :], ALU.min, ALU.mult
            )
        else:
            # scale = exp(-0.5*relu(ln(ss))) == min(1, rsqrt(ss)), exact at 1.
            nc.scalar.activation(ta[:, :], tb[:, :], AF.Ln)
            nc.scalar.activation(tb[:, :], ta[:, :], AF.Relu)
            sc = sp.tile([P, W // 3], F32, name="sc", tag="sc")[:, :r]
            nc.scalar.activation(sc[:, :], tb[:, :], AF.Exp, scale=-0.5)
            # m_k = d_k * scale, into the sq tile (squares are dead now).
            if fuse_mul:
                scb = sc.broadcast_to([P, r, 3])
                nc.vector.tensor_mul(m3[:, :, :], d3[:, :, :], scb)
            else:
                for k in range(3):
                    nc.vector.tensor_mul(m3[:, :, k], d3[:, :, k], sc[:, :])

        # out = m + c, into the d tile (d is dead now).
        if add_split > 0.0:
            w2 = int(w * (1.0 - add_split)) // 96 * 96
            if w2 > 0:
                nc.gpsimd.tensor_add(dt[:, :w2], sq[:, :w2], ct[:, :w2])
            nc.vector.tensor_add(dt[:, w2:], sq[:, w2:], ct[:, w2:])
        else:
            nc.gpsimd.tensor_add(dt[:, :], sq[:, :], ct[:, :])

        if variant != "compute_only":
            _emit_out(nc, o_dma, orr[:, off : off + w], dt[:, :], w)
        off += w


_NC = None

# Final config (measured ~127 us/core steady-state vs ~118 us pure-DMA floor):
# 16 chunks of 1536 floats/partition, 6-deep pools, rsqrt algo, final add on
# DVE (GPSIMD fully serializes against DVE on this part — no compute there),
# out-DMA on the ACT HWDGE ring.
_SCHEDULE = [1536] * 16
_BUILD_KWARGS = dict(
    schedule=_SCHEDULE, W=1536, bufs=6,
    algo="rsqrt", add_split=1.0, out_ring="act",
)


def _get_nc():
    global _NC
    if _NC is None:
        _NC = _build(**_BUILD_KWARGS)
    return _NC


def kernel(**inputs):
    x = np.asarray(inputs["x"], dtype=np.float32)
    center = np.asarray(inputs["center"], dtype=np.float32)
    assert x.shape == (B, 3) and center.shape == (B, 3)

    xs = x.reshape(N_CORES, B_CORE, 3)
    cs = center.reshape(N_CORES, B_CORE, 3)
    in_maps = [
        {"x": np.ascontiguousarray(xs[i]), "center": np.ascontiguousarray(cs[i])}
        for i in range(N_CORES)
    ]

    nc = _get_nc()
    res = run_bass_kernel_spmd(nc, in_maps, list(range(N_CORES)))
    out = np.concatenate([res.results[i]["out"] for i in range(N_CORES)], axis=0)
    return out.astype(np.float32, copy=False)


if __name__ == "__main__":
    nc = _get_nc()
    print("build ok")
